# revision 7
# baseline (speedup 1.0000x reference)
"""Distributed causal multi-head attention for Trainium2 (8 NeuronCores).

Problem: B=2, S=2048, NX=1024, H=16 heads, D=64.
  qkv = x @ w_attn + b_attn ; q,k,v split; causal softmax(q k^T / 8) v ; @ w_proj + b_proj

Sharding: core c -> batch b=c//4 (data parallel), head group g=c%4 (tensor
parallel, 4 heads). Column-split c_attn, row-split c_proj; partial outputs
are summed with per-chunk ReduceScatter (overlapped with compute) and the
host reassembles the shards.

Layout strategy: host passes x transposed (xT [NX, S]) so QKV projections,
scores and PV products all run in matmul-native layouts with zero on-chip
transposes. Scores are computed transposed ([k, q]): softmax reduction over
k then lands on the partition axis, where an extra ones-column appended to V
yields the denominator for free in the same PV matmul. exp() needs no
max-subtraction (scores are bounded; ACT exp is <=2 ULP on [-10,10]).
All matmuls run in float32r (4x fp32 PE throughput, ~1.5e-4 rel err).
"""

import sys

sys.path.insert(0, "/opt/trn_rl_repo")

import numpy as np

B = 2
S = 2048
NX = 1024
H = 16
D = 64
G = 4            # head groups (tensor-parallel)
HL = H // G      # heads per core = 4
HDW = HL * D     # head-group width = 256
P = 128
SC = 512         # sequence chunk (queries per chunk)
NQC = S // SC    # 4 chunks
NE = NX // P     # 8 contraction tiles
NKB = S // P     # 16 key blocks
REPLICA_GROUPS = [[0, 1, 2, 3], [4, 5, 6, 7]]

_COMPILED = None


def _build():
    import concourse.bass as bass  # noqa: F401
    import concourse.mybir as mybir
    import concourse.tile as tile
    from concourse import bacc

    f32 = mybir.dt.float32
    f32r = mybir.dt.float32r
    Identity = mybir.ActivationFunctionType.Identity
    Exp = mybir.ActivationFunctionType.Exp

    nc = bacc.Bacc("TRN2", target_bir_lowering=False, debug=False, num_devices=8)

    xT = nc.dram_tensor("xT", [NX, S], f32, kind="ExternalInput")
    wqk = nc.dram_tensor("wqk", [NX, 2 * HDW], f32, kind="ExternalInput")
    wv = nc.dram_tensor("wv", [NX, HDW], f32, kind="ExternalInput")
    wp = nc.dram_tensor("wp", [HDW, NX], f32, kind="ExternalInput")
    bqk = nc.dram_tensor("bqk", [4, P], f32, kind="ExternalInput")
    bv = nc.dram_tensor("bv", [1, HDW], f32, kind="ExternalInput")
    bp = nc.dram_tensor("bp", [1, NX], f32, kind="ExternalInput")
    causalT = nc.dram_tensor("causalT", [P, P], f32, kind="ExternalInput")
    onesc = nc.dram_tensor("onesc", [P, 4], f32, kind="ExternalInput")
    out_ext = nc.dram_tensor("out", [NQC, SC // 4, NX], f32, kind="ExternalOutput")

    with tile.TileContext(nc) as tc:
        with (
            tc.tile_pool(name="const", bufs=1) as const_pool,
            tc.tile_pool(name="xt", bufs=1) as xt_pool,
            tc.tile_pool(name="w", bufs=1) as w_pool,
            tc.tile_pool(name="qkt", bufs=1) as qkt_pool,
            tc.tile_pool(name="vsb", bufs=1) as v_pool,
            tc.tile_pool(name="atsb", bufs=1) as at_pool,
            tc.tile_pool(name="exp", bufs=6) as exp_pool,
            tc.tile_pool(name="osb", bufs=3) as osb_pool,
            tc.tile_pool(name="small", bufs=2) as small_pool,
            tc.tile_pool(name="mmps", bufs=2, space="PSUM") as mm_ps,
            tc.tile_pool(name="scps", bufs=3, space="PSUM") as sc_ps_pool,
            tc.tile_pool(name="atps", bufs=2, space="PSUM") as at_ps_pool,
            tc.tile_pool(name="rbps", bufs=1, space="PSUM") as rb_ps_pool,
            tc.tile_pool(name="dram", bufs=1, space="DRAM") as dram_pool,
        ):
            # ---- constant / weight loads ----
            bqk_sb = const_pool.tile([P, 4], f32, name="bqk_sb")
            for fi in range(4):
                nc.sync.dma_start(bqk_sb[:, fi : fi + 1], bqk[fi : fi + 1, :])
            bv_sb = const_pool.tile([1, HDW], f32r, name="bv_sb")
            nc.sync.dma_start(bv_sb[:], bv[:].bitcast(f32r))
            bp_sb = const_pool.tile([1, NX], f32r, name="bp_sb")
            nc.sync.dma_start(bp_sb[:], bp[:].bitcast(f32r))
            cz_sb = const_pool.tile([P, P], f32, name="cz_sb")
            nc.sync.dma_start(cz_sb[:], causalT[:])
            ones1 = const_pool.tile([1, P], f32r, name="ones1")
            nc.sync.dma_start(ones1[:], onesc[:, 0:1].bitcast(f32r))

            wqk_sb = []
            for e in range(NE):
                t = w_pool.tile([P, 2 * HDW], f32r, name=f"wqk_sb{e}")
                nc.sync.dma_start(t[:], wqk[e * P : (e + 1) * P, :].bitcast(f32r))
                wqk_sb.append(t)
            wv_sb = []
            for e in range(NE):
                t = w_pool.tile([P, HDW], f32r, name=f"wv_sb{e}")
                nc.sync.dma_start(t[:], wv[e * P : (e + 1) * P, :].bitcast(f32r))
                wv_sb.append(t)
            wp_sb = []
            for kt in range(2):
                t = w_pool.tile([P, NX], f32r, name=f"wp_sb{kt}")
                nc.sync.dma_start(t[:], wp[kt * P : (kt + 1) * P, :].bitcast(f32r))
                wp_sb.append(t)

            # xT tiles split by (e, sc) so compute can start before the full
            # 8MB xT load lands.
            xt_sb = {}
            for e in range(NE):
                for sc in range(NQC):
                    t = xt_pool.tile([P, SC], f32r, name=f"xt{e}_{sc}")
                    nc.sync.dma_start(
                        t[:],
                        xT[e * P : (e + 1) * P, sc * SC : (sc + 1) * SC].bitcast(f32r),
                    )
                    xt_sb[e, sc] = t

            # ---- phase 1: qkT [2*HDW, S] and v [S, HDW (+ones)] ----
            # qkT tile (fi, sc): partitions = feature dim (q: fi 0..1, k: fi 2..3)
            qkt_sb = {}
            v_sb = {}
            for sc in range(NQC):
                for fi in range(4):
                    ps = mm_ps.tile([P, SC], f32, tag="mm", name=f"qk_ps{fi}_{sc}")
                    for e in range(NE):
                        nc.tensor.matmul(
                            ps[:],
                            wqk_sb[e][:, fi * P : (fi + 1) * P],
                            xt_sb[e, sc][:],
                            start=(e == 0),
                            stop=(e == NE - 1),
                        )
                    t = qkt_pool.tile([P, SC], f32r, name=f"qkt{fi}_{sc}")
                    # fold the 1/sqrt(D)=1/8 score scale into q (bias comes
                    # pre-scaled from the host)
                    scale = 0.125 if fi < 2 else 1.0
                    nc.scalar.activation(
                        t[:], ps[:], Identity, bias=bqk_sb[:, fi : fi + 1], scale=scale
                    )
                    qkt_sb[fi, sc] = t
                for j in range(4):
                    si = 4 * sc + j
                    psv = mm_ps.tile([P, HDW], f32, tag="mm", name=f"v_ps{si}")
                    for e in range(NE):
                        nc.tensor.matmul(
                            psv[:],
                            xt_sb[e, sc][:, j * P : (j + 1) * P],
                            wv_sb[e][:],
                            start=(e == 0),
                            stop=(e == NE - 1),
                        )
                    # v block + ones column per head: [128, 4*(64+1)]
                    vt = v_pool.tile([P, HL * (D + 1)], f32r, name=f"v{si}")
                    nc.sync.dma_start(
                        vt[:].rearrange("p (h u) -> p h u", h=HL)[:, :, D : D + 1],
                        onesc[:].bitcast(f32r),
                    )
                    nc.scalar.activation(
                        vt[:].rearrange("p (h u) -> p h u", h=HL)[:, :, 0:D],
                        psv[:].rearrange("p (h u) -> p h u", h=HL),
                        Identity,
                    )
                    v_sb[si] = vt

            # ---- phase 2: attention + projection + ReduceScatter, per chunk ----
            at_sb = {}  # (kt, qc) -> [128, SC] f32r; partitions = head dims
            for qc in range(NQC):
                for h in range(HL):
                    fi_q = h // 2
                    fi_k = 2 + h // 2
                    prow = 64 * (h % 2)
                    at_ps = at_ps_pool.tile(
                        [D + 1, SC], f32, tag="atps", name=f"at_ps{qc}_{h}"
                    )
                    kmax = 4 * qc + 3
                    for kb in range(kmax + 1):
                        off = P * max(0, kb - 4 * qc)
                        nn = SC - off
                        scp = sc_ps_pool.tile(
                            [P, SC], f32, tag="sc", name=f"sc_ps{qc}_{h}_{kb}"
                        )
                        # scoresT[k, q] = kT^T qT (contraction over d)
                        nc.tensor.matmul(
                            scp[:, 0:nn],
                            qkt_sb[fi_k, kb // 4][
                                prow : prow + D, (kb % 4) * P : (kb % 4 + 1) * P
                            ],
                            qkt_sb[fi_q, qc][prow : prow + D, off:SC],
                            start=True,
                            stop=True,
                        )
                        if kb >= 4 * qc:
                            # diagonal block: additive causal mask
                            nc.vector.tensor_add(
                                scp[:, 0:P], scp[:, 0:P], cz_sb[:]
                            )
                        ex = exp_pool.tile([P, SC], f32r, tag="exp", name=f"ex{qc}_{h}_{kb}")
                        nc.scalar.activation(ex[:, 0:nn], scp[:, 0:nn], Exp)
                        # PV (+denominator via ones column of v)
                        nc.tensor.matmul(
                            at_ps[:, off : off + nn],
                            v_sb[kb][:, h * (D + 1) : (h + 1) * (D + 1)],
                            ex[:, 0:nn],
                            start=(kb == 0),
                            stop=False,
                        )
                    dn = small_pool.tile([1, SC], f32r, tag="dn", name=f"dn{qc}_{h}")
                    nc.vector.tensor_copy(dn[:], at_ps[D : D + 1, :])
                    # bv folded in as a rank-1 update: (PV + bv denom^T)/denom
                    nc.tensor.matmul(
                        at_ps[0:D, :],
                        bv_sb[0:1, h * D : (h + 1) * D],
                        dn[:],
                        start=False,
                        stop=True,
                    )
                    rc32 = small_pool.tile([1, SC], f32, tag="rc32", name=f"rc32{qc}_{h}")
                    nc.vector.reciprocal(rc32[:], dn[:])
                    rc = small_pool.tile([1, SC], f32r, tag="rc", name=f"rc{qc}_{h}")
                    nc.vector.tensor_copy(rc[:], rc32[:])
                    # broadcast 1/denom across partitions via K=1 matmul
                    rb = rb_ps_pool.tile([D, SC], f32, tag="rb", name=f"rb{qc}_{h}")
                    nc.tensor.matmul(rb[:], ones1[0:1, 0:D], rc[:], start=True, stop=True)
                    rb_sb = small_pool.tile([D, SC], f32, tag="rbsb", name=f"rbsb{qc}_{h}")
                    nc.vector.tensor_copy(rb_sb[:], rb[:])
                    kt = h // 2
                    if (kt, qc) not in at_sb:
                        at_sb[kt, qc] = at_pool.tile([P, SC], f32r, name=f"at{kt}_{qc}")
                    nc.vector.tensor_mul(
                        at_sb[kt, qc][prow : prow + D, :], at_ps[0:D, :], rb_sb[:]
                    )

                # c_proj partial for this chunk + chunked ReduceScatter
                op_dram = dram_pool.tile([SC, NX], f32, tag=f"op{qc}", name=f"op{qc}")
                for st in range(4):
                    o_sb = osb_pool.tile([P, NX], f32, tag="osb", name=f"osb{qc}_{st}")
                    for nn2 in range(2):
                        pp = mm_ps.tile([P, SC], f32, tag="mm", name=f"pj_ps{qc}_{st}_{nn2}")
                        for kt in range(2):
                            nc.tensor.matmul(
                                pp[:],
                                at_sb[kt, qc][:, st * P : (st + 1) * P],
                                wp_sb[kt][:, nn2 * SC : (nn2 + 1) * SC],
                                start=(kt == 0),
                                stop=False,
                            )
                        # bias via K=1 ones matmul (only group-leader core has
                        # nonzero bp, so the group sum adds it exactly once)
                        nc.tensor.matmul(
                            pp[:],
                            ones1[0:1, :],
                            bp_sb[0:1, nn2 * SC : (nn2 + 1) * SC],
                            start=False,
                            stop=True,
                        )
                        nc.vector.tensor_copy(o_sb[:, nn2 * SC : (nn2 + 1) * SC], pp[:])
                    nc.sync.dma_start(op_dram[st * P : (st + 1) * P, :], o_sb[:])
                rs_out = dram_pool.tile(
                    [SC // 4, NX], f32, tag=f"rs{qc}", name=f"rs{qc}"
                )
                nc.gpsimd.collective_compute(
                    "ReduceScatter",
                    mybir.AluOpType.add,
                    ins=[op_dram[:].opt()],
                    outs=[rs_out[:].opt()],
                    replica_groups=REPLICA_GROUPS,
                )
                nc.sync.dma_start(out_ext[qc], rs_out[:])

    nc.compile()
    return nc


def _get_compiled():
    global _COMPILED
    if _COMPILED is None:
        _COMPILED = _build()
    return _COMPILED


def make_in_maps(x, attention_mask, w_attn, b_attn, w_proj, b_proj):
    x = np.asarray(x, dtype=np.float32)
    w_attn = np.asarray(w_attn, dtype=np.float32)
    b_attn = np.asarray(b_attn, dtype=np.float32)
    w_proj = np.asarray(w_proj, dtype=np.float32)
    b_proj = np.asarray(b_proj, dtype=np.float32)

    ki, qi = np.meshgrid(np.arange(P), np.arange(P), indexing="ij")
    causalT = np.where(ki > qi, np.float32(-1e9), np.float32(0.0))

    in_maps = []
    for c in range(8):
        b, g = divmod(c, 4)
        cols = slice(HDW * g, HDW * (g + 1))
        kcols = slice(NX + HDW * g, NX + HDW * (g + 1))
        vcols = slice(2 * NX + HDW * g, 2 * NX + HDW * (g + 1))
        bqk = np.concatenate([b_attn[cols] * 0.125, b_attn[kcols]]).reshape(4, P)
        in_maps.append(
            {
                "xT": np.ascontiguousarray(x[b].T),
                "wqk": np.ascontiguousarray(
                    np.concatenate([w_attn[:, cols], w_attn[:, kcols]], axis=1)
                ),
                "wv": np.ascontiguousarray(w_attn[:, vcols]),
                "wp": np.ascontiguousarray(w_proj[HDW * g : HDW * (g + 1), :]),
                "bqk": np.ascontiguousarray(bqk),
                "bv": np.ascontiguousarray(b_attn[vcols].reshape(1, HDW)),
                "bp": np.ascontiguousarray(
                    (b_proj if g == 0 else np.zeros_like(b_proj)).reshape(1, NX)
                ),
                "causalT": causalT,
                "onesc": np.ones((P, 4), dtype=np.float32),
            }
        )
    return in_maps


def assemble_out(results):
    out = np.empty((B, S, NX), dtype=np.float32)
    for c in range(8):
        b, g = divmod(c, 4)
        chunk = results[c]["out"]  # [NQC, 128, NX]
        for qc in range(NQC):
            r0 = qc * SC + g * (SC // 4)
            out[b, r0 : r0 + SC // 4, :] = chunk[qc]
    return out


def run(in_maps, trace=False):
    from concourse.bass_utils import run_bass_kernel_spmd

    nc = _get_compiled()
    return run_bass_kernel_spmd(nc, in_maps, core_ids=list(range(8)), trace=trace)


def kernel(**inputs) -> np.ndarray:
    in_maps = make_in_maps(**inputs)
    res = run(in_maps)
    return assemble_out(res.results)


if __name__ == "__main__":
    _get_compiled()
    print("build+compile OK")


# revision 16
# speedup vs baseline: 1.0670x; 1.0670x over previous
"""Distributed causal multi-head attention for Trainium2 (8 NeuronCores).

Problem: B=2, S=2048, NX=1024, H=16 heads, D=64.
  qkv = x @ w_attn + b_attn ; q,k,v split; causal softmax(q k^T / 8) v ; @ w_proj + b_proj

Sharding: core c -> batch b=c//4 (data parallel), head group g=c%4 (tensor
parallel, 4 heads). Column-split c_attn; after attention an intra-group
AllToAll reshards heads->sequence so each core computes c_proj for its own
512 output rows with the full hidden dim - no cross-core reduction and only
2MB/rank on the wire.

Layout strategy: host passes x transposed (xT [NX, S]) so QKV projections,
scores and PV products all run in matmul-native layouts with zero on-chip
transposes. Scores are computed transposed ([k, q]): the softmax reduction
over k lands on the partition axis, where an extra ones-column appended to V
yields the denominator for free in the same PV matmul. exp() needs no
max-subtraction (scores are bounded; ACT exp is <=2 ULP on [-10,10]).
All matmuls run in float32r (4x fp32 PE throughput, ~1.5e-4 rel err).
"""

import sys

sys.path.insert(0, "/opt/trn_rl_repo")

import numpy as np

B = 2
S = 2048
NX = 1024
H = 16
D = 64
G = 4            # head groups (tensor-parallel)
HL = H // G      # heads per core = 4
HDW = HL * D     # head-group width = 256
P = 128
SC = 512         # sequence chunk (queries per chunk)
NQC = S // SC    # 4 chunks
NE = NX // P     # 8 contraction tiles
NKB = S // P     # 16 key blocks
REPLICA_GROUPS = [[0, 1, 2, 3], [4, 5, 6, 7]]

_COMPILED = None


def _build():
    import concourse.bass as bass  # noqa: F401
    import concourse.mybir as mybir
    import concourse.tile as tile
    from concourse import bacc

    f32 = mybir.dt.float32
    f32r = mybir.dt.float32r
    Identity = mybir.ActivationFunctionType.Identity
    Exp = mybir.ActivationFunctionType.Exp

    nc = bacc.Bacc("TRN2", target_bir_lowering=False, debug=False, num_devices=8)

    xT = nc.dram_tensor("xT", [NX, S], f32, kind="ExternalInput")
    wqk = nc.dram_tensor("wqk", [NX, 2 * HDW], f32, kind="ExternalInput")
    wv = nc.dram_tensor("wv", [NX, HDW], f32, kind="ExternalInput")
    wp = nc.dram_tensor("wp", [2 * NX, NX], f32, kind="ExternalInput")
    bqk = nc.dram_tensor("bqk", [4, P], f32, kind="ExternalInput")
    bv = nc.dram_tensor("bv", [1, HDW], f32, kind="ExternalInput")
    bp = nc.dram_tensor("bp", [1, NX], f32, kind="ExternalInput")
    causalT = nc.dram_tensor("causalT", [P, P], f32, kind="ExternalInput")
    onesc = nc.dram_tensor("onesc", [P, 4], f32, kind="ExternalInput")
    out_ext = nc.dram_tensor("out", [SC, NX], f32, kind="ExternalOutput")

    with tile.TileContext(nc) as tc:
        with (
            tc.tile_pool(name="const", bufs=1) as const_pool,
            tc.tile_pool(name="xt", bufs=1) as xt_pool,
            tc.tile_pool(name="w", bufs=1) as w_pool,
            tc.tile_pool(name="qkt", bufs=1) as qkt_pool,
            tc.tile_pool(name="vsb", bufs=1) as v_pool,
            tc.tile_pool(name="atsb", bufs=1) as at_pool,
            tc.tile_pool(name="exp", bufs=4) as exp_pool,
            tc.tile_pool(name="osb", bufs=2) as osb_pool,
            tc.tile_pool(name="small", bufs=2) as small_pool,
            tc.tile_pool(name="mmps", bufs=2, space="PSUM") as mm_ps,
            tc.tile_pool(name="scps", bufs=3, space="PSUM") as sc_ps_pool,
            tc.tile_pool(name="atps", bufs=2, space="PSUM") as at_ps_pool,
            tc.tile_pool(name="rbps", bufs=1, space="PSUM") as rb_ps_pool,
            tc.tile_pool(name="dram", bufs=1, space="DRAM") as dram_pool,
        ):
            # ---- constants ----
            bqk_sb = const_pool.tile([P, 4], f32, name="bqk_sb")
            for fi in range(4):
                nc.sync.dma_start(bqk_sb[:, fi : fi + 1], bqk[fi : fi + 1, :])
            bv_sb = const_pool.tile([1, HDW], f32r, name="bv_sb")
            nc.sync.dma_start(bv_sb[:], bv[:].bitcast(f32r))
            bp_sb = const_pool.tile([1, NX], f32r, name="bp_sb")
            nc.sync.dma_start(bp_sb[:], bp[:].bitcast(f32r))
            cz_sb = const_pool.tile([P, P], f32, name="cz_sb")
            nc.sync.dma_start(cz_sb[:], causalT[:])
            ones1 = const_pool.tile([1, P], f32r, name="ones1")
            nc.sync.dma_start(ones1[:], onesc[:, 0:1].bitcast(f32r))

            # ---- weight + xT loads (first-needed first) ----
            wqk_sb = []
            xt_sb = {}
            for e in range(NE):
                t = w_pool.tile([P, 2 * HDW], f32r, name=f"wqk_sb{e}")
                nc.sync.dma_start(t[:], wqk[e * P : (e + 1) * P, :].bitcast(f32r))
                wqk_sb.append(t)
                t2 = xt_pool.tile([P, SC], f32r, name=f"xt{e}_0", tag=f"xts{e}_0")
                nc.sync.dma_start(
                    t2[:], xT[e * P : (e + 1) * P, 0:SC].bitcast(f32r)
                )
                xt_sb[e, 0] = t2
            wv_sb = []
            for e in range(NE):
                t = w_pool.tile([P, HDW], f32r, name=f"wv_sb{e}")
                nc.sync.dma_start(t[:], wv[e * P : (e + 1) * P, :].bitcast(f32r))
                wv_sb.append(t)
            for sc in range(1, NQC):
                for e in range(NE):
                    t = xt_pool.tile([P, SC], f32r, name=f"xt{e}_{sc}", tag=f"xts{e}_{sc}")
                    nc.sync.dma_start(
                        t[:],
                        xT[e * P : (e + 1) * P, sc * SC : (sc + 1) * SC].bitcast(f32r),
                    )
                    xt_sb[e, sc] = t

            # extended (junk-masked) w_proj tiles, loaded late into recycled
            # xT slots (xt slot (e, sc) frees once phase 1 consumed it)
            wp_sb = {}
            for kt2 in range(2 * NE):
                for nn2 in range(2):
                    t = xt_pool.tile(
                        [P, SC], f32r, name=f"wp{kt2}_{nn2}",
                        tag=f"xts{kt2 % NE}_{(kt2 // NE) * 2 + nn2}",
                    )
                    nc.sync.dma_start(
                        t[:],
                        wp[kt2 * P : (kt2 + 1) * P, nn2 * SC : (nn2 + 1) * SC].bitcast(
                            f32r
                        ),
                    )
                    wp_sb[kt2, nn2] = t

            # ---- phase 1: qkT [2*HDW, S] and v [S, HDW (+ones)] ----
            qkt_sb = {}
            v_sb = {}
            for sc in range(NQC):
                for fi in range(4):
                    ps = mm_ps.tile([P, SC], f32, tag="mm", name=f"qk_ps{fi}_{sc}")
                    for e in range(NE):
                        nc.tensor.matmul(
                            ps[:],
                            wqk_sb[e][:, fi * P : (fi + 1) * P],
                            xt_sb[e, sc][:],
                            start=(e == 0),
                            stop=(e == NE - 1),
                        )
                    t = qkt_pool.tile(
                        [P, SC], f32r, name=f"qkt{fi}_{sc}", tag=f"qkts{fi}_{sc}"
                    )
                    # fold the 1/sqrt(D)=1/8 score scale into q (bias comes
                    # pre-scaled from the host)
                    scale = 0.125 if fi < 2 else 1.0
                    nc.scalar.activation(
                        t[:], ps[:], Identity, bias=bqk_sb[:, fi : fi + 1], scale=scale
                    )
                    qkt_sb[fi, sc] = t
                for j in range(4):
                    si = 4 * sc + j
                    psv = mm_ps.tile([P, HDW], f32, tag="mm", name=f"v_ps{si}")
                    for e in range(NE):
                        nc.tensor.matmul(
                            psv[:],
                            xt_sb[e, sc][:, j * P : (j + 1) * P],
                            wv_sb[e][:],
                            start=(e == 0),
                            stop=(e == NE - 1),
                        )
                    vt = v_pool.tile([P, HL * (D + 1)], f32r, name=f"v{si}")
                    nc.sync.dma_start(
                        vt[:].rearrange("p (h u) -> p h u", h=HL)[:, :, D : D + 1],
                        onesc[:].bitcast(f32r),
                    )
                    nc.scalar.activation(
                        vt[:].rearrange("p (h u) -> p h u", h=HL)[:, :, 0:D],
                        psv[:].rearrange("p (h u) -> p h u", h=HL),
                        Identity,
                    )
                    v_sb[si] = vt

            # ---- phase 2: attention (scoresT -> exp -> PV), chunked over qc ----
            # A2A buffer has 8 rank chunks; chunk j carries our aT for
            # s-range j%4 (both batch positions get the same data; the
            # receiver zeroes the other batch's rows via wp).
            a2a_in = dram_pool.tile([8, 2 * P, SC], f32, tag="a2a_in", name="a2a_in")
            at_sb = {}  # (kt, qc) -> [128, SC] f32; partitions = head dims

            def make_tail(qc, h, at_ps):
                """Normalize head (qc, h); emitted one head late so the PE-side
                broadcast matmul never stalls on the DVE reciprocal chain."""

                def tail():
                    prow = 64 * (h % 2)
                    kt = h // 2
                    dn_r = small_pool.tile([1, SC], f32r, tag="dnr", name=f"dnr{qc}_{h}")
                    nc.vector.tensor_copy(dn_r[:], at_ps[D : D + 1, :])
                    # bv folded in as a rank-1 update: (PV + bv denom^T)/denom
                    nc.tensor.matmul(
                        at_ps[0:D, :],
                        bv_sb[0:1, h * D : (h + 1) * D],
                        dn_r[:],
                        start=False,
                        stop=True,
                    )
                    dn32 = small_pool.tile([1, SC], f32, tag="dn32", name=f"dn32{qc}_{h}")
                    nc.vector.tensor_copy(dn32[:], at_ps[D : D + 1, :])
                    rc32 = small_pool.tile([1, SC], f32, tag="rc32", name=f"rc32{qc}_{h}")
                    scr = small_pool.tile([1, SC], f32, tag="scr", name=f"scr{qc}_{h}")
                    nc.vector.reciprocal_approx_accurate(rc32[:], dn32[:], scr[:])
                    rc = small_pool.tile([1, SC], f32r, tag="rc", name=f"rc{qc}_{h}")
                    nc.vector.tensor_copy(rc[:], rc32[:])
                    # broadcast 1/denom across partitions via K=1 matmul
                    rb = rb_ps_pool.tile([D, SC], f32, tag="rb", name=f"rb{qc}_{h}")
                    nc.tensor.matmul(rb[:], ones1[0:1, 0:D], rc[:], start=True, stop=True)
                    rb_sb = small_pool.tile([D, SC], f32, tag="rbsb", name=f"rbsb{qc}_{h}")
                    nc.vector.tensor_copy(rb_sb[:], rb[:])
                    if (kt, qc) not in at_sb:
                        at_sb[kt, qc] = at_pool.tile([P, SC], f32, name=f"at{kt}_{qc}")
                    nc.vector.tensor_mul(
                        at_sb[kt, qc][prow : prow + D, :], at_ps[0:D, :], rb_sb[:]
                    )
                    if h % 2 == 1:
                        # both heads of this kt landed -> ship to the A2A buffer
                        nc.sync.dma_start(
                            a2a_in[qc, kt * P : (kt + 1) * P, :], at_sb[kt, qc][:]
                        )
                        nc.sync.dma_start(
                            a2a_in[qc + 4, kt * P : (kt + 1) * P, :], at_sb[kt, qc][:]
                        )

                return tail

            pending = None
            for qc in range(NQC):
                for h in range(HL):
                    fi_q = h // 2
                    fi_k = 2 + h // 2
                    prow = 64 * (h % 2)
                    at_ps = at_ps_pool.tile(
                        [D + 1, SC], f32, tag="atps", name=f"at_ps{qc}_{h}"
                    )
                    for kb in range(4 * qc + 4):
                        off = P * max(0, kb - 4 * qc)
                        nn = SC - off
                        scp = sc_ps_pool.tile(
                            [P, SC], f32, tag="sc", name=f"sc_ps{qc}_{h}_{kb}"
                        )
                        # scoresT[k, q] = kT^T qT (contraction over d)
                        nc.tensor.matmul(
                            scp[:, 0:nn],
                            qkt_sb[fi_k, kb // 4][
                                prow : prow + D, (kb % 4) * P : (kb % 4 + 1) * P
                            ],
                            qkt_sb[fi_q, qc][prow : prow + D, off:SC],
                            start=True,
                            stop=True,
                        )
                        if kb >= 4 * qc:
                            # diagonal block: additive causal mask
                            nc.vector.tensor_add(scp[:, 0:P], scp[:, 0:P], cz_sb[:])
                        ex = exp_pool.tile(
                            [P, SC], f32r, tag="exp", name=f"ex{qc}_{h}_{kb}"
                        )
                        nc.scalar.activation(ex[:, 0:nn], scp[:, 0:nn], Exp)
                        # PV (+denominator via the ones column of v)
                        nc.tensor.matmul(
                            at_ps[:, off : off + nn],
                            v_sb[kb][:, h * (D + 1) : (h + 1) * (D + 1)],
                            ex[:, 0:nn],
                            start=(kb == 0),
                            stop=False,
                        )
                    if pending is not None:
                        pending()
                    pending = make_tail(qc, h, at_ps)
            pending()

            # ---- phase 3: AllToAll (heads -> sequence) + c_proj ----
            a2a_out = dram_pool.tile([8, 2 * P, SC], f32, tag="a2a_out", name="a2a_out")
            nc.gpsimd.collective_compute(
                "AllToAll",
                mybir.AluOpType.bypass,
                ins=[a2a_in[:].opt()],
                outs=[a2a_out[:].opt()],
                replica_groups=[list(range(8))],
            )
            # chunk j = core j's heads for our s-range; the other batch's
            # chunks are neutralized by zero rows in wp (host-prepared)
            lh_sb = {}
            for kt2 in range(2 * NE):
                t = qkt_pool.tile(
                    [P, SC], f32r, name=f"lh{kt2}",
                    tag=f"qkts{kt2 % 4}_{kt2 // 4}",
                )
                nc.sync.dma_start(
                    t[:],
                    a2a_out[kt2 // 2, (kt2 % 2) * P : (kt2 % 2 + 1) * P, :].bitcast(f32r),
                )
                lh_sb[kt2] = t
            for st in range(4):
                o_sb = osb_pool.tile([P, NX], f32, tag="osb", name=f"osb{st}")
                for nn2 in range(2):
                    pp = mm_ps.tile([P, SC], f32, tag="mm", name=f"pj_ps{st}_{nn2}")
                    for kt2 in range(2 * NE):
                        nc.tensor.matmul(
                            pp[:],
                            lh_sb[kt2][:, st * P : (st + 1) * P],
                            wp_sb[kt2, nn2][:],
                            start=(kt2 == 0),
                            stop=False,
                        )
                    # + b_proj via K=1 ones matmul (each core owns distinct rows)
                    nc.tensor.matmul(
                        pp[:],
                        ones1[0:1, :],
                        bp_sb[0:1, nn2 * SC : (nn2 + 1) * SC],
                        start=False,
                        stop=True,
                    )
                    nc.vector.tensor_copy(o_sb[:, nn2 * SC : (nn2 + 1) * SC], pp[:])
                nc.sync.dma_start(out_ext[st * P : (st + 1) * P, :], o_sb[:])

    nc.compile()
    return nc


def _get_compiled():
    global _COMPILED
    if _COMPILED is None:
        _COMPILED = _build()
    return _COMPILED


def make_in_maps(x, attention_mask, w_attn, b_attn, w_proj, b_proj):
    x = np.asarray(x, dtype=np.float32)
    w_attn = np.asarray(w_attn, dtype=np.float32)
    b_attn = np.asarray(b_attn, dtype=np.float32)
    w_proj = np.asarray(w_proj, dtype=np.float32)
    b_proj = np.asarray(b_proj, dtype=np.float32)

    ki, qi = np.meshgrid(np.arange(P), np.arange(P), indexing="ij")
    causalT = np.where(ki > qi, np.float32(-1e9), np.float32(0.0))
    xTs = [np.ascontiguousarray(x[b].T) for b in range(B)]
    # extended w_proj: rows [1024*b : 1024*(b+1)] hold the real w_proj, the
    # other batch's rows are zero (masks that batch's A2A chunks)
    wp_ext = []
    for b in range(B):
        w = np.zeros((2 * NX, NX), dtype=np.float32)
        w[NX * b : NX * (b + 1), :] = w_proj
        wp_ext.append(w)
    bp_row = np.ascontiguousarray(b_proj.reshape(1, NX))

    in_maps = []
    for c in range(8):
        b, g = divmod(c, 4)
        cols = slice(HDW * g, HDW * (g + 1))
        kcols = slice(NX + HDW * g, NX + HDW * (g + 1))
        vcols = slice(2 * NX + HDW * g, 2 * NX + HDW * (g + 1))
        bqk = np.concatenate([b_attn[cols] * 0.125, b_attn[kcols]]).reshape(4, P)
        in_maps.append(
            {
                "xT": xTs[b],
                "wqk": np.ascontiguousarray(
                    np.concatenate([w_attn[:, cols], w_attn[:, kcols]], axis=1)
                ),
                "wv": np.ascontiguousarray(w_attn[:, vcols]),
                "wp": wp_ext[b],
                "bqk": np.ascontiguousarray(bqk),
                "bv": np.ascontiguousarray(b_attn[vcols].reshape(1, HDW)),
                "bp": bp_row,
                "causalT": causalT,
                "onesc": np.ones((P, 4), dtype=np.float32),
            }
        )
    return in_maps


def assemble_out(results):
    out = np.empty((B, S, NX), dtype=np.float32)
    for c in range(8):
        b, g = divmod(c, 4)
        out[b, g * SC : (g + 1) * SC, :] = results[c]["out"]
    return out


def run(in_maps, trace=False):
    from concourse.bass_utils import run_bass_kernel_spmd

    nc = _get_compiled()
    return run_bass_kernel_spmd(nc, in_maps, core_ids=list(range(8)), trace=trace)


def kernel(**inputs) -> np.ndarray:
    in_maps = make_in_maps(**inputs)
    res = run(in_maps)
    return assemble_out(res.results)


if __name__ == "__main__":
    _get_compiled()
    print("build+compile OK")


# revision 17
# speedup vs baseline: 1.2020x; 1.1266x over previous
"""Distributed causal multi-head attention for Trainium2 (8 NeuronCores).

Problem: B=2, S=2048, NX=1024, H=16 heads, D=64.
  qkv = x @ w_attn + b_attn ; q,k,v split; causal softmax(q k^T / 8) v ; @ w_proj + b_proj

Sharding: core c -> batch b=c//4 (data parallel), head group g=c%4 (tensor
parallel, 4 heads). Column-split c_attn; after attention an intra-group
AllToAll reshards heads->sequence so each core computes c_proj for its own
512 output rows with the full hidden dim - no cross-core reduction and only
2MB/rank on the wire.

Layout strategy: host passes x transposed (xT [NX, S]) so QKV projections,
scores and PV products all run in matmul-native layouts with zero on-chip
transposes. Scores are computed transposed ([k, q]): the softmax reduction
over k lands on the partition axis, where an extra ones-column appended to V
yields the denominator for free in the same PV matmul. exp() needs no
max-subtraction (scores are bounded; ACT exp is <=2 ULP on [-10,10]).
All matmuls run in float32r (4x fp32 PE throughput, ~1.5e-4 rel err).
"""

import sys

sys.path.insert(0, "/opt/trn_rl_repo")

import numpy as np

B = 2
S = 2048
NX = 1024
H = 16
D = 64
G = 4            # head groups (tensor-parallel)
HL = H // G      # heads per core = 4
HDW = HL * D     # head-group width = 256
P = 128
SC = 512         # sequence chunk (queries per chunk)
NQC = S // SC    # 4 chunks
NE = NX // P     # 8 contraction tiles
NKB = S // P     # 16 key blocks
REPLICA_GROUPS = [[0, 1, 2, 3], [4, 5, 6, 7]]

_COMPILED = None


def _build():
    import concourse.bass as bass  # noqa: F401
    import concourse.mybir as mybir
    import concourse.tile as tile
    from concourse import bacc

    f32 = mybir.dt.float32
    f32r = mybir.dt.float32r
    Identity = mybir.ActivationFunctionType.Identity
    Exp = mybir.ActivationFunctionType.Exp

    nc = bacc.Bacc("TRN2", target_bir_lowering=False, debug=False, num_devices=8)

    xT = nc.dram_tensor("xT", [NX, S], f32, kind="ExternalInput")
    wqk = nc.dram_tensor("wqk", [NX, 2 * HDW], f32, kind="ExternalInput")
    wv = nc.dram_tensor("wv", [NX, HDW], f32, kind="ExternalInput")
    wp = nc.dram_tensor("wp", [2 * NX, NX], f32, kind="ExternalInput")
    bqk = nc.dram_tensor("bqk", [4, P], f32, kind="ExternalInput")
    bv = nc.dram_tensor("bv", [1, HDW], f32, kind="ExternalInput")
    bp = nc.dram_tensor("bp", [1, NX], f32, kind="ExternalInput")
    causalT = nc.dram_tensor("causalT", [P, P], f32, kind="ExternalInput")
    onesc = nc.dram_tensor("onesc", [P, 4], f32, kind="ExternalInput")
    out_ext = nc.dram_tensor("out", [SC, NX], f32, kind="ExternalOutput")

    with tile.TileContext(nc) as tc:
        with (
            tc.tile_pool(name="const", bufs=1) as const_pool,
            tc.tile_pool(name="xt", bufs=1) as xt_pool,
            tc.tile_pool(name="w", bufs=1) as w_pool,
            tc.tile_pool(name="qkt", bufs=1) as qkt_pool,
            tc.tile_pool(name="vsb", bufs=1) as v_pool,
            tc.tile_pool(name="atsb", bufs=1) as at_pool,
            tc.tile_pool(name="exp", bufs=4) as exp_pool,
            tc.tile_pool(name="osb", bufs=2) as osb_pool,
            tc.tile_pool(name="small", bufs=2) as small_pool,
            tc.tile_pool(name="mmps", bufs=2, space="PSUM") as mm_ps,
            tc.tile_pool(name="scps", bufs=3, space="PSUM") as sc_ps_pool,
            tc.tile_pool(name="atps", bufs=2, space="PSUM") as at_ps_pool,
            tc.tile_pool(name="rbps", bufs=1, space="PSUM") as rb_ps_pool,
            tc.tile_pool(name="dram", bufs=1, space="DRAM") as dram_pool,
        ):
            # ---- constants ----
            bqk_sb = const_pool.tile([P, 4], f32, name="bqk_sb")
            for fi in range(4):
                nc.sync.dma_start(bqk_sb[:, fi : fi + 1], bqk[fi : fi + 1, :])
            bv_sb = const_pool.tile([1, HDW], f32r, name="bv_sb")
            nc.sync.dma_start(bv_sb[:], bv[:].bitcast(f32r))
            bp_sb = const_pool.tile([1, NX], f32r, name="bp_sb")
            nc.sync.dma_start(bp_sb[:], bp[:].bitcast(f32r))
            cz_sb = const_pool.tile([P, P], f32, name="cz_sb")
            nc.sync.dma_start(cz_sb[:], causalT[:])
            ones1 = const_pool.tile([1, P], f32r, name="ones1")
            nc.sync.dma_start(ones1[:], onesc[:, 0:1].bitcast(f32r))

            # ---- weight + xT loads (first-needed first) ----
            wqk_sb = []
            xt_sb = {}
            for e in range(NE):
                t = w_pool.tile([P, 2 * HDW], f32r, name=f"wqk_sb{e}")
                nc.sync.dma_start(t[:], wqk[e * P : (e + 1) * P, :].bitcast(f32r))
                wqk_sb.append(t)
                t2 = xt_pool.tile([P, SC], f32r, name=f"xt{e}_0", tag=f"xts{e}_0")
                nc.sync.dma_start(
                    t2[:], xT[e * P : (e + 1) * P, 0:SC].bitcast(f32r)
                )
                xt_sb[e, 0] = t2
            wv_sb = []
            for e in range(NE):
                t = w_pool.tile([P, HDW], f32r, name=f"wv_sb{e}")
                nc.sync.dma_start(t[:], wv[e * P : (e + 1) * P, :].bitcast(f32r))
                wv_sb.append(t)
            for sc in range(1, NQC):
                for e in range(NE):
                    t = xt_pool.tile([P, SC], f32r, name=f"xt{e}_{sc}", tag=f"xts{e}_{sc}")
                    nc.sync.dma_start(
                        t[:],
                        xT[e * P : (e + 1) * P, sc * SC : (sc + 1) * SC].bitcast(f32r),
                    )
                    xt_sb[e, sc] = t

            # extended (junk-masked) w_proj tiles, loaded late into recycled
            # xT slots (xt slot (e, sc) frees once phase 1 consumed it)
            wp_sb = {}
            for kt2 in range(2 * NE):
                for nn2 in range(2):
                    t = xt_pool.tile(
                        [P, SC], f32r, name=f"wp{kt2}_{nn2}",
                        tag=f"xts{kt2 % NE}_{(kt2 // NE) * 2 + nn2}",
                    )
                    nc.sync.dma_start(
                        t[:],
                        wp[kt2 * P : (kt2 + 1) * P, nn2 * SC : (nn2 + 1) * SC].bitcast(
                            f32r
                        ),
                    )
                    wp_sb[kt2, nn2] = t

            # ---- phase 1: qkT [2*HDW, S] and v [S, HDW (+ones)] ----
            qkt_sb = {}
            v_sb = {}
            for sc in range(NQC):
                for fi in range(4):
                    ps = mm_ps.tile([P, SC], f32, tag="mm", name=f"qk_ps{fi}_{sc}")
                    for e in range(NE):
                        nc.tensor.matmul(
                            ps[:],
                            wqk_sb[e][:, fi * P : (fi + 1) * P],
                            xt_sb[e, sc][:],
                            start=(e == 0),
                            stop=(e == NE - 1),
                        )
                    t = qkt_pool.tile(
                        [P, SC], f32r, name=f"qkt{fi}_{sc}", tag=f"qkts{fi}_{sc}"
                    )
                    # fold the 1/sqrt(D)=1/8 score scale into q (bias comes
                    # pre-scaled from the host)
                    scale = 0.125 if fi < 2 else 1.0
                    nc.scalar.activation(
                        t[:], ps[:], Identity, bias=bqk_sb[:, fi : fi + 1], scale=scale
                    )
                    qkt_sb[fi, sc] = t
                for j in range(4):
                    si = 4 * sc + j
                    psv = mm_ps.tile([P, HDW], f32, tag="mm", name=f"v_ps{si}")
                    for e in range(NE):
                        nc.tensor.matmul(
                            psv[:],
                            xt_sb[e, sc][:, j * P : (j + 1) * P],
                            wv_sb[e][:],
                            start=(e == 0),
                            stop=(e == NE - 1),
                        )
                    vt = v_pool.tile([P, HL * (D + 1)], f32r, name=f"v{si}")
                    nc.sync.dma_start(
                        vt[:].rearrange("p (h u) -> p h u", h=HL)[:, :, D : D + 1],
                        onesc[:].bitcast(f32r),
                    )
                    nc.scalar.activation(
                        vt[:].rearrange("p (h u) -> p h u", h=HL)[:, :, 0:D],
                        psv[:].rearrange("p (h u) -> p h u", h=HL),
                        Identity,
                    )
                    v_sb[si] = vt

            # ---- phase 2: attention (scoresT -> exp -> PV) ----
            # Head-major order: head h's A2A (one per head, [8, 64, SC])
            # fires as soon as head h is done over all chunks and overlaps
            # with head h+1's attention. Each rank chunk j carries our
            # head-h rows for s-range j%4 (both batch positions get the same
            # data; the receiver zeroes the other batch's rows via wp).
            a2a_in = {}
            a2a_out = {}
            for h in range(HL):
                a2a_in[h] = dram_pool.tile(
                    [8, D, SC], f32, tag=f"a2a_in{h}", name=f"a2a_in{h}"
                )
                a2a_out[h] = dram_pool.tile(
                    [8, D, SC], f32, tag=f"a2a_out{h}", name=f"a2a_out{h}"
                )

            def make_tail(h, qc, at_ps):
                """Normalize head (h, qc); emitted one chunk late so the
                PE-side broadcast matmul never stalls on the DVE reciprocal
                chain."""

                def tail():
                    dn_r = small_pool.tile([1, SC], f32r, tag="dnr", name=f"dnr{qc}_{h}")
                    nc.vector.tensor_copy(dn_r[:], at_ps[D : D + 1, :])
                    # bv folded in as a rank-1 update: (PV + bv denom^T)/denom
                    nc.tensor.matmul(
                        at_ps[0:D, :],
                        bv_sb[0:1, h * D : (h + 1) * D],
                        dn_r[:],
                        start=False,
                        stop=True,
                    )
                    dn32 = small_pool.tile([1, SC], f32, tag="dn32", name=f"dn32{qc}_{h}")
                    nc.vector.tensor_copy(dn32[:], at_ps[D : D + 1, :])
                    rc32 = small_pool.tile([1, SC], f32, tag="rc32", name=f"rc32{qc}_{h}")
                    scr = small_pool.tile([1, SC], f32, tag="scr", name=f"scr{qc}_{h}")
                    nc.vector.reciprocal_approx_accurate(rc32[:], dn32[:], scr[:])
                    rc = small_pool.tile([1, SC], f32r, tag="rc", name=f"rc{qc}_{h}")
                    nc.vector.tensor_copy(rc[:], rc32[:])
                    # broadcast 1/denom across partitions via K=1 matmul
                    rb = rb_ps_pool.tile([D, SC], f32, tag="rb", name=f"rb{qc}_{h}")
                    nc.tensor.matmul(rb[:], ones1[0:1, 0:D], rc[:], start=True, stop=True)
                    rb_sb = small_pool.tile([D, SC], f32, tag="rbsb", name=f"rbsb{qc}_{h}")
                    nc.vector.tensor_copy(rb_sb[:], rb[:])
                    ath = small_pool.tile([D, SC], f32, tag="ath", bufs=3, name=f"ath{qc}_{h}")
                    nc.vector.tensor_mul(ath[:], at_ps[0:D, :], rb_sb[:])
                    nc.sync.dma_start(a2a_in[h][qc], ath[:])
                    nc.sync.dma_start(a2a_in[h][qc + 4], ath[:])

                return tail

            pending = None
            pending_hqc = None
            for h in range(HL):
                fi_q = h // 2
                fi_k = 2 + h // 2
                prow = 64 * (h % 2)
                for qc in range(NQC):
                    at_ps = at_ps_pool.tile(
                        [D + 1, SC], f32, tag="atps", name=f"at_ps{qc}_{h}"
                    )
                    for kb in range(4 * qc + 4):
                        off = P * max(0, kb - 4 * qc)
                        nn = SC - off
                        scp = sc_ps_pool.tile(
                            [P, SC], f32, tag="sc", name=f"sc_ps{qc}_{h}_{kb}"
                        )
                        # scoresT[k, q] = kT^T qT (contraction over d)
                        nc.tensor.matmul(
                            scp[:, 0:nn],
                            qkt_sb[fi_k, kb // 4][
                                prow : prow + D, (kb % 4) * P : (kb % 4 + 1) * P
                            ],
                            qkt_sb[fi_q, qc][prow : prow + D, off:SC],
                            start=True,
                            stop=True,
                        )
                        if kb >= 4 * qc:
                            # diagonal block: additive causal mask
                            nc.vector.tensor_add(scp[:, 0:P], scp[:, 0:P], cz_sb[:])
                        ex = exp_pool.tile(
                            [P, SC], f32r, tag="exp", name=f"ex{qc}_{h}_{kb}"
                        )
                        nc.scalar.activation(ex[:, 0:nn], scp[:, 0:nn], Exp)
                        # PV (+denominator via the ones column of v)
                        nc.tensor.matmul(
                            at_ps[:, off : off + nn],
                            v_sb[kb][:, h * (D + 1) : (h + 1) * (D + 1)],
                            ex[:, 0:nn],
                            start=(kb == 0),
                            stop=False,
                        )
                    if pending is not None:
                        pending()
                        if pending_hqc[1] == NQC - 1:
                            # previous head fully normalized -> its A2A
                            hprev = pending_hqc[0]
                            nc.gpsimd.collective_compute(
                                "AllToAll",
                                mybir.AluOpType.bypass,
                                ins=[a2a_in[hprev][:].opt()],
                                outs=[a2a_out[hprev][:].opt()],
                                replica_groups=[list(range(8))],
                            )
                    pending = make_tail(h, qc, at_ps)
                    pending_hqc = (h, qc)
            pending()
            nc.gpsimd.collective_compute(
                "AllToAll",
                mybir.AluOpType.bypass,
                ins=[a2a_in[HL - 1][:].opt()],
                outs=[a2a_out[HL - 1][:].opt()],
                replica_groups=[list(range(8))],
            )

            # ---- phase 3: assemble gathered activations + c_proj ----
            # extended-hd row 128*kt2 .. +128 = source core j = kt2//2,
            # local heads (2*(kt2%2), 2*(kt2%2)+1)
            lh_sb = {}
            for kt2 in range(2 * NE):
                t = qkt_pool.tile(
                    [P, SC], f32r, name=f"lh{kt2}",
                    tag=f"qkts{kt2 % 4}_{kt2 // 4}",
                )
                j = kt2 // 2
                for hh in range(2):
                    hloc = 2 * (kt2 % 2) + hh
                    nc.sync.dma_start(
                        t[hh * D : (hh + 1) * D, :],
                        a2a_out[hloc][j].bitcast(f32r),
                    )
                lh_sb[kt2] = t
            for st in range(4):
                o_sb = osb_pool.tile([P, NX], f32, tag="osb", name=f"osb{st}")
                for nn2 in range(2):
                    pp = mm_ps.tile([P, SC], f32, tag="mm", name=f"pj_ps{st}_{nn2}")
                    for kt2 in range(2 * NE):
                        nc.tensor.matmul(
                            pp[:],
                            lh_sb[kt2][:, st * P : (st + 1) * P],
                            wp_sb[kt2, nn2][:],
                            start=(kt2 == 0),
                            stop=False,
                        )
                    # + b_proj via K=1 ones matmul (each core owns distinct rows)
                    nc.tensor.matmul(
                        pp[:],
                        ones1[0:1, :],
                        bp_sb[0:1, nn2 * SC : (nn2 + 1) * SC],
                        start=False,
                        stop=True,
                    )
                    nc.vector.tensor_copy(o_sb[:, nn2 * SC : (nn2 + 1) * SC], pp[:])
                nc.sync.dma_start(out_ext[st * P : (st + 1) * P, :], o_sb[:])

    nc.compile()
    return nc


def _get_compiled():
    global _COMPILED
    if _COMPILED is None:
        _COMPILED = _build()
    return _COMPILED


def make_in_maps(x, attention_mask, w_attn, b_attn, w_proj, b_proj):
    x = np.asarray(x, dtype=np.float32)
    w_attn = np.asarray(w_attn, dtype=np.float32)
    b_attn = np.asarray(b_attn, dtype=np.float32)
    w_proj = np.asarray(w_proj, dtype=np.float32)
    b_proj = np.asarray(b_proj, dtype=np.float32)

    ki, qi = np.meshgrid(np.arange(P), np.arange(P), indexing="ij")
    causalT = np.where(ki > qi, np.float32(-1e9), np.float32(0.0))
    xTs = [np.ascontiguousarray(x[b].T) for b in range(B)]
    # extended w_proj: rows [1024*b : 1024*(b+1)] hold the real w_proj, the
    # other batch's rows are zero (masks that batch's A2A chunks)
    wp_ext = []
    for b in range(B):
        w = np.zeros((2 * NX, NX), dtype=np.float32)
        w[NX * b : NX * (b + 1), :] = w_proj
        wp_ext.append(w)
    bp_row = np.ascontiguousarray(b_proj.reshape(1, NX))

    in_maps = []
    for c in range(8):
        b, g = divmod(c, 4)
        cols = slice(HDW * g, HDW * (g + 1))
        kcols = slice(NX + HDW * g, NX + HDW * (g + 1))
        vcols = slice(2 * NX + HDW * g, 2 * NX + HDW * (g + 1))
        bqk = np.concatenate([b_attn[cols] * 0.125, b_attn[kcols]]).reshape(4, P)
        in_maps.append(
            {
                "xT": xTs[b],
                "wqk": np.ascontiguousarray(
                    np.concatenate([w_attn[:, cols], w_attn[:, kcols]], axis=1)
                ),
                "wv": np.ascontiguousarray(w_attn[:, vcols]),
                "wp": wp_ext[b],
                "bqk": np.ascontiguousarray(bqk),
                "bv": np.ascontiguousarray(b_attn[vcols].reshape(1, HDW)),
                "bp": bp_row,
                "causalT": causalT,
                "onesc": np.ones((P, 4), dtype=np.float32),
            }
        )
    return in_maps


def assemble_out(results):
    out = np.empty((B, S, NX), dtype=np.float32)
    for c in range(8):
        b, g = divmod(c, 4)
        out[b, g * SC : (g + 1) * SC, :] = results[c]["out"]
    return out


def run(in_maps, trace=False):
    from concourse.bass_utils import run_bass_kernel_spmd

    nc = _get_compiled()
    return run_bass_kernel_spmd(nc, in_maps, core_ids=list(range(8)), trace=trace)


def kernel(**inputs) -> np.ndarray:
    in_maps = make_in_maps(**inputs)
    res = run(in_maps)
    return assemble_out(res.results)


if __name__ == "__main__":
    _get_compiled()
    print("build+compile OK")


# revision 21
# speedup vs baseline: 1.3452x; 1.1191x over previous
"""Distributed causal multi-head attention for Trainium2 (8 NeuronCores).

Problem: B=2, S=2048, NX=1024, H=16 heads, D=64.
  qkv = x @ w_attn + b_attn ; q,k,v split; causal softmax(q k^T / 8) v ; @ w_proj + b_proj

Sharding: core c -> batch b=c//4 (data parallel), head group g=c%4 (tensor
parallel, 4 heads). Column-split c_attn; after attention an intra-group
AllToAll reshards heads->sequence so each core computes c_proj for its own
512 output rows with the full hidden dim - no cross-core reduction and only
2MB/rank on the wire.

Layout strategy: host passes x transposed (xT [NX, S]) so QKV projections,
scores and PV products all run in matmul-native layouts with zero on-chip
transposes. Scores are computed transposed ([k, q]): the softmax reduction
over k lands on the partition axis, where an extra ones-column appended to V
yields the denominator for free in the same PV matmul. exp() needs no
max-subtraction (scores are bounded; ACT exp is <=2 ULP on [-10,10]).
All matmuls run in float32r (4x fp32 PE throughput, ~1.5e-4 rel err).
"""

import sys

sys.path.insert(0, "/opt/trn_rl_repo")

import numpy as np
import ml_dtypes

BF16 = ml_dtypes.bfloat16

B = 2
S = 2048
NX = 1024
H = 16
D = 64
G = 4            # head groups (tensor-parallel)
HL = H // G      # heads per core = 4
HDW = HL * D     # head-group width = 256
P = 128
SC = 512         # sequence chunk (queries per chunk)
NQC = S // SC    # 4 chunks
NE = NX // P     # 8 contraction tiles
NKB = S // P     # 16 key blocks
REPLICA_GROUPS = [[0, 1, 2, 3], [4, 5, 6, 7]]

_COMPILED = None
import os
ATTN_F32R = os.environ.get('ATTN_F32R', '0') == '1'
PROJ_F32R = os.environ.get('PROJ_F32R', '0') == '1'


def _build():
    import concourse.bass as bass  # noqa: F401
    import concourse.mybir as mybir
    import concourse.tile as tile
    from concourse import bacc

    f32 = mybir.dt.float32
    f32r = mybir.dt.float32r
    bf16 = mybir.dt.bfloat16
    adt = f32r if ATTN_F32R else bf16
    pdt = f32r if PROJ_F32R else bf16
    AW = (D + 1) if ATTN_F32R else P  # v slot width per head
    Identity = mybir.ActivationFunctionType.Identity
    Exp = mybir.ActivationFunctionType.Exp

    nc = bacc.Bacc("TRN2", target_bir_lowering=False, debug=False, num_devices=8)

    xT = nc.dram_tensor("xT", [NX, S], bf16, kind="ExternalInput")
    wqk = nc.dram_tensor("wqk", [NX, 2 * HDW], bf16, kind="ExternalInput")
    wv = nc.dram_tensor("wv", [NX, HDW], bf16, kind="ExternalInput")
    wp = nc.dram_tensor("wp", [2 * NX, NX], f32 if PROJ_F32R else bf16, kind="ExternalInput")
    bqk = nc.dram_tensor("bqk", [4, P], f32, kind="ExternalInput")
    bv = nc.dram_tensor("bv", [1, HDW], f32, kind="ExternalInput")
    bp = nc.dram_tensor("bp", [1, NX], f32, kind="ExternalInput")
    causalT = nc.dram_tensor("causalT", [P, P], f32, kind="ExternalInput")
    onesc = nc.dram_tensor("onesc", [P, 4], f32, kind="ExternalInput")
    onesb = nc.dram_tensor("onesb", [P, 4], bf16, kind="ExternalInput")
    out_ext = nc.dram_tensor("out", [SC, NX], f32, kind="ExternalOutput")

    with tile.TileContext(nc) as tc:
        with (
            tc.tile_pool(name="const", bufs=1) as const_pool,
            tc.tile_pool(name="xt", bufs=1) as xt_pool,
            tc.tile_pool(name="w", bufs=1) as w_pool,
            tc.tile_pool(name="qkt", bufs=1) as qkt_pool,
            tc.tile_pool(name="vsb", bufs=1) as v_pool,
            tc.tile_pool(name="atsb", bufs=1) as at_pool,
            tc.tile_pool(name="exp", bufs=4) as exp_pool,
            tc.tile_pool(name="osb", bufs=2) as osb_pool,
            tc.tile_pool(name="small", bufs=2) as small_pool,
            tc.tile_pool(name="mmps", bufs=2, space="PSUM") as mm_ps,
            tc.tile_pool(name="scps", bufs=3, space="PSUM") as sc_ps_pool,
            tc.tile_pool(name="atps", bufs=2, space="PSUM") as at_ps_pool,
            tc.tile_pool(name="rbps", bufs=1, space="PSUM") as rb_ps_pool,
            tc.tile_pool(name="dram", bufs=1, space="DRAM") as dram_pool,
        ):
            # ---- constants ----
            bqk_sb = const_pool.tile([P, 4], f32, name="bqk_sb")
            for fi in range(4):
                nc.sync.dma_start(bqk_sb[:, fi : fi + 1], bqk[fi : fi + 1, :])
            bv_sb = const_pool.tile([1, HDW], f32r, name="bv_sb")
            nc.sync.dma_start(bv_sb[:], bv[:].bitcast(f32r))
            bp_sb = const_pool.tile([1, NX], f32r, name="bp_sb")
            nc.sync.dma_start(bp_sb[:], bp[:].bitcast(f32r))
            cz_sb = const_pool.tile([P, P], f32, name="cz_sb")
            nc.sync.dma_start(cz_sb[:], causalT[:])
            ones1 = const_pool.tile([1, P], f32r, name="ones1")
            nc.sync.dma_start(ones1[:], onesc[:, 0:1].bitcast(f32r))

            # ---- weight + xT loads (first-needed first) ----
            wqk_sb = []
            xt_sb = {}
            for e in range(NE):
                t = w_pool.tile([P, 2 * HDW], bf16, name=f"wqk_sb{e}")
                nc.sync.dma_start(t[:], wqk[e * P : (e + 1) * P, :])
                wqk_sb.append(t)
                t2 = xt_pool.tile([P, SC], bf16, name=f"xt{e}_0", tag=f"xts{e}_0")
                nc.sync.dma_start(t2[:], xT[e * P : (e + 1) * P, 0:SC])
                xt_sb[e, 0] = t2
            wv_sb = []
            for e in range(NE):
                t = w_pool.tile([P, HDW], bf16, name=f"wv_sb{e}")
                nc.sync.dma_start(t[:], wv[e * P : (e + 1) * P, :])
                wv_sb.append(t)
            for sc in range(1, NQC):
                for e in range(NE):
                    t = xt_pool.tile([P, SC], bf16, name=f"xt{e}_{sc}", tag=f"xts{e}_{sc}")
                    nc.sync.dma_start(
                        t[:],
                        xT[e * P : (e + 1) * P, sc * SC : (sc + 1) * SC],
                    )
                    xt_sb[e, sc] = t

            # extended (junk-masked) w_proj tiles, loaded late into recycled
            # xT slots (xt slot (e, sc) frees once phase 1 consumed it)
            wp_sb = {}
            for kt2 in range(2 * NE):
                for nn2 in range(2):
                    t = xt_pool.tile(
                        [P, SC], pdt, name=f"wp{kt2}_{nn2}",
                        tag=f"xts{kt2 % NE}_{(kt2 // NE) * 2 + nn2}",
                    )
                    src_ap = wp[kt2 * P : (kt2 + 1) * P, nn2 * SC : (nn2 + 1) * SC]
                    if PROJ_F32R:
                        nc.gpsimd.dma_start(t[:], src_ap)
                    else:
                        nc.sync.dma_start(t[:], src_ap)
                    wp_sb[kt2, nn2] = t

            # ---- phase 1: qkT [2*HDW, S] and v [S, HDW (+ones)] ----
            qkt_sb = {}
            v_sb = {}
            for sc in range(NQC):
                for fi in range(4):
                    ps = mm_ps.tile([P, SC], f32, tag="mm", name=f"qk_ps{fi}_{sc}")
                    for e in range(NE):
                        nc.tensor.matmul(
                            ps[:],
                            wqk_sb[e][:, fi * P : (fi + 1) * P],
                            xt_sb[e, sc][:],
                            start=(e == 0),
                            stop=(e == NE - 1),
                        )
                    t = qkt_pool.tile(
                        [P, SC], adt, name=f"qkt{fi}_{sc}", tag=f"qkts{fi}_{sc}"
                    )
                    # fold the 1/sqrt(D)=1/8 score scale into q (bias comes
                    # pre-scaled from the host)
                    scale = 0.125 if fi < 2 else 1.0
                    nc.scalar.activation(
                        t[:], ps[:], Identity, bias=bqk_sb[:, fi : fi + 1], scale=scale
                    )
                    qkt_sb[fi, sc] = t
                for j in range(4):
                    si = 4 * sc + j
                    psv = mm_ps.tile([P, HDW], f32, tag="mm", name=f"v_ps{si}")
                    for e in range(NE):
                        nc.tensor.matmul(
                            psv[:],
                            xt_sb[e, sc][:, j * P : (j + 1) * P],
                            wv_sb[e][:],
                            start=(e == 0),
                            stop=(e == NE - 1),
                        )
                    vt = v_pool.tile([P, HL * AW], adt, name=f"v{si}")
                    if ATTN_F32R:
                        nc.sync.dma_start(
                            vt[:].rearrange("p (h u) -> p h u", h=HL)[:, :, D : D + 1],
                            onesc[:].bitcast(f32r),
                        )
                    else:
                        nc.vector.memset(vt[:], 0.0)
                        nc.sync.dma_start(
                            vt[:].rearrange("p (h u) -> p h u", h=HL)[:, :, D : D + 1],
                            onesb[:],
                        )
                    nc.scalar.activation(
                        vt[:].rearrange("p (h u) -> p h u", h=HL)[:, :, 0:D],
                        psv[:].rearrange("p (h u) -> p h u", h=HL),
                        Identity,
                    )
                    v_sb[si] = vt

            # ---- phase 2: attention (scoresT -> exp -> PV) ----
            # Head-major order: head h's A2A (one per head, [8, 64, SC])
            # fires as soon as head h is done over all chunks and overlaps
            # with head h+1's attention. Each rank chunk j carries our
            # head-h rows for s-range j%4 (both batch positions get the same
            # data; the receiver zeroes the other batch's rows via wp).
            a2a_in = {}
            a2a_out = {}
            for h in range(HL):
                a2a_in[h] = dram_pool.tile(
                    [8, D, SC], f32 if PROJ_F32R else bf16, tag=f"a2a_in{h}", name=f"a2a_in{h}"
                )
                a2a_out[h] = dram_pool.tile(
                    [8, D, SC], f32 if PROJ_F32R else bf16, tag=f"a2a_out{h}", name=f"a2a_out{h}"
                )

            def make_tail(h, qc, at_ps):
                """Normalize head (h, qc); emitted one chunk late so the
                PE-side broadcast matmul never stalls on the DVE reciprocal
                chain."""

                def tail():
                    dn_r = small_pool.tile([1, SC], f32r, tag="dnr", name=f"dnr{qc}_{h}")
                    nc.vector.tensor_copy(dn_r[:], at_ps[D : D + 1, :])
                    # bv folded in as a rank-1 update: (PV + bv denom^T)/denom
                    nc.tensor.matmul(
                        at_ps[0:D, :],
                        bv_sb[0:1, h * D : (h + 1) * D],
                        dn_r[:],
                        start=False,
                        stop=True,
                    )
                    dn32 = small_pool.tile([1, SC], f32, tag="dn32", name=f"dn32{qc}_{h}")
                    nc.vector.tensor_copy(dn32[:], at_ps[D : D + 1, :])
                    rc32 = small_pool.tile([1, SC], f32, tag="rc32", name=f"rc32{qc}_{h}")
                    scr = small_pool.tile([1, SC], f32, tag="scr", name=f"scr{qc}_{h}")
                    nc.vector.reciprocal_approx_accurate(rc32[:], dn32[:], scr[:])
                    rc = small_pool.tile([1, SC], f32r, tag="rc", name=f"rc{qc}_{h}")
                    nc.vector.tensor_copy(rc[:], rc32[:])
                    # broadcast 1/denom across partitions via K=1 matmul
                    rb = rb_ps_pool.tile([D, SC], f32, tag="rb", name=f"rb{qc}_{h}")
                    nc.tensor.matmul(rb[:], ones1[0:1, 0:D], rc[:], start=True, stop=True)
                    rb_sb = small_pool.tile([D, SC], f32, tag="rbsb", name=f"rbsb{qc}_{h}")
                    nc.vector.tensor_copy(rb_sb[:], rb[:])
                    ath = small_pool.tile([D, SC], f32 if PROJ_F32R else bf16, tag="ath", bufs=3, name=f"ath{qc}_{h}")
                    nc.vector.tensor_mul(ath[:], at_ps[0:D, :], rb_sb[:])
                    nc.sync.dma_start(a2a_in[h][qc], ath[:])
                    nc.sync.dma_start(a2a_in[h][qc + 4], ath[:])

                return tail

            pending = None
            pending_hqc = None
            for h in range(HL):
                fi_q = h // 2
                fi_k = 2 + h // 2
                prow = 64 * (h % 2)
                for qc in range(NQC):
                    at_ps = at_ps_pool.tile(
                        [AW, SC], f32, tag="atps", name=f"at_ps{qc}_{h}"
                    )
                    for kb in range(4 * qc + 4):
                        off = P * max(0, kb - 4 * qc)
                        nn = SC - off
                        scp = sc_ps_pool.tile(
                            [P, SC], f32, tag="sc", name=f"sc_ps{qc}_{h}_{kb}"
                        )
                        # scoresT[k, q] = kT^T qT (contraction over d)
                        nc.tensor.matmul(
                            scp[:, 0:nn],
                            qkt_sb[fi_k, kb // 4][
                                prow : prow + D, (kb % 4) * P : (kb % 4 + 1) * P
                            ],
                            qkt_sb[fi_q, qc][prow : prow + D, off:SC],
                            start=True,
                            stop=True,
                        )
                        if kb >= 4 * qc:
                            # diagonal block: additive causal mask
                            nc.vector.tensor_add(scp[:, 0:P], scp[:, 0:P], cz_sb[:])
                        ex = exp_pool.tile(
                            [P, SC], adt, tag="exp", name=f"ex{qc}_{h}_{kb}"
                        )
                        nc.scalar.activation(ex[:, 0:nn], scp[:, 0:nn], Exp)
                        # PV (+denominator via the ones column of v)
                        nc.tensor.matmul(
                            at_ps[:, off : off + nn],
                            v_sb[kb][:, h * AW : (h + 1) * AW],
                            ex[:, 0:nn],
                            start=(kb == 0),
                            stop=False,
                        )
                    if pending is not None:
                        pending()
                        if pending_hqc[1] == NQC - 1:
                            # previous head fully normalized -> its A2A
                            hprev = pending_hqc[0]
                            nc.gpsimd.collective_compute(
                                "AllToAll",
                                mybir.AluOpType.bypass,
                                ins=[a2a_in[hprev][:].opt()],
                                outs=[a2a_out[hprev][:].opt()],
                                replica_groups=[list(range(8))],
                            )
                    pending = make_tail(h, qc, at_ps)
                    pending_hqc = (h, qc)
            pending()
            nc.gpsimd.collective_compute(
                "AllToAll",
                mybir.AluOpType.bypass,
                ins=[a2a_in[HL - 1][:].opt()],
                outs=[a2a_out[HL - 1][:].opt()],
                replica_groups=[list(range(8))],
            )

            # ---- phase 3: assemble gathered activations + c_proj ----
            # extended-hd row 128*kt2 .. +128 = source core j = kt2//2,
            # local heads (2*(kt2%2), 2*(kt2%2)+1)
            lh_sb = {}
            for kt2 in range(2 * NE):
                t = qkt_pool.tile(
                    [P, SC], pdt, name=f"lh{kt2}",
                    tag=f"qkts{kt2 % 4}_{kt2 // 4}",
                )
                j = kt2 // 2
                for hh in range(2):
                    hloc = 2 * (kt2 % 2) + hh
                    src_ap = a2a_out[hloc][j]
                    if PROJ_F32R:
                        nc.sync.dma_start(t[hh * D : (hh + 1) * D, :], src_ap.bitcast(f32r))
                    else:
                        nc.sync.dma_start(t[hh * D : (hh + 1) * D, :], src_ap)
                lh_sb[kt2] = t
            for st in range(4):
                o_sb = osb_pool.tile([P, NX], f32, tag="osb", name=f"osb{st}")
                for nn2 in range(2):
                    pp = mm_ps.tile([P, SC], f32, tag="mm", name=f"pj_ps{st}_{nn2}")
                    for kt2 in range(2 * NE):
                        nc.tensor.matmul(
                            pp[:],
                            lh_sb[kt2][:, st * P : (st + 1) * P],
                            wp_sb[kt2, nn2][:],
                            start=(kt2 == 0),
                            stop=False,
                        )
                    # + b_proj via K=1 ones matmul (each core owns distinct rows)
                    nc.tensor.matmul(
                        pp[:],
                        ones1[0:1, :],
                        bp_sb[0:1, nn2 * SC : (nn2 + 1) * SC],
                        start=False,
                        stop=True,
                    )
                    nc.vector.tensor_copy(o_sb[:, nn2 * SC : (nn2 + 1) * SC], pp[:])
                nc.sync.dma_start(out_ext[st * P : (st + 1) * P, :], o_sb[:])

    nc.compile()
    return nc


def _get_compiled():
    global _COMPILED
    if _COMPILED is None:
        _COMPILED = _build()
    return _COMPILED


def make_in_maps(x, attention_mask, w_attn, b_attn, w_proj, b_proj):
    x = np.asarray(x, dtype=np.float32)
    w_attn = np.asarray(w_attn, dtype=np.float32)
    b_attn = np.asarray(b_attn, dtype=np.float32)
    w_proj = np.asarray(w_proj, dtype=np.float32)
    b_proj = np.asarray(b_proj, dtype=np.float32)

    ki, qi = np.meshgrid(np.arange(P), np.arange(P), indexing="ij")
    causalT = np.where(ki > qi, np.float32(-1e9), np.float32(0.0))
    xTs = [np.ascontiguousarray(x[b].T.astype(BF16)) for b in range(B)]
    # extended w_proj: rows [1024*b : 1024*(b+1)] hold the real w_proj, the
    # other batch's rows are zero (masks that batch's A2A chunks)
    import os as _os
    _wpdt = np.float32 if _os.environ.get('PROJ_F32R', '0') == '1' else BF16
    wp_ext = []
    for b in range(B):
        w = np.zeros((2 * NX, NX), dtype=_wpdt)
        w[NX * b : NX * (b + 1), :] = w_proj.astype(_wpdt)
        wp_ext.append(w)
    bp_row = np.ascontiguousarray(b_proj.reshape(1, NX))

    in_maps = []
    for c in range(8):
        b, g = divmod(c, 4)
        cols = slice(HDW * g, HDW * (g + 1))
        kcols = slice(NX + HDW * g, NX + HDW * (g + 1))
        vcols = slice(2 * NX + HDW * g, 2 * NX + HDW * (g + 1))
        bqk = np.concatenate([b_attn[cols] * 0.125, b_attn[kcols]]).reshape(4, P)
        in_maps.append(
            {
                "xT": xTs[b],
                "wqk": np.ascontiguousarray(
                    np.concatenate([w_attn[:, cols], w_attn[:, kcols]], axis=1).astype(
                        BF16
                    )
                ),
                "wv": np.ascontiguousarray(w_attn[:, vcols].astype(BF16)),
                "wp": wp_ext[b],
                "bqk": np.ascontiguousarray(bqk),
                "bv": np.ascontiguousarray(b_attn[vcols].reshape(1, HDW)),
                "bp": bp_row,
                "causalT": causalT,
                "onesc": np.ones((P, 4), dtype=np.float32),
                "onesb": np.ones((P, 4), dtype=BF16),
            }
        )
    return in_maps


def assemble_out(results):
    out = np.empty((B, S, NX), dtype=np.float32)
    for c in range(8):
        b, g = divmod(c, 4)
        out[b, g * SC : (g + 1) * SC, :] = results[c]["out"]
    return out


def run(in_maps, trace=False):
    from concourse.bass_utils import run_bass_kernel_spmd

    nc = _get_compiled()
    return run_bass_kernel_spmd(nc, in_maps, core_ids=list(range(8)), trace=trace)


def kernel(**inputs) -> np.ndarray:
    in_maps = make_in_maps(**inputs)
    res = run(in_maps)
    return assemble_out(res.results)


if __name__ == "__main__":
    _get_compiled()
    print("build+compile OK")
